# revision 7
# baseline (speedup 1.0000x reference)
"""Trainium2 Bass kernel for packed varlen causal attention (8 seqs x 1024 tok).

Sharding: data-parallel over sequences -- core i computes sequence i end to end.
Weights are replicated. No collectives.

Device-side math (per core, S=1024 tokens, E=1024, H=16, D=64):
  QT[e,t] = (0.125*Wq)^T-matmul, + 0.125*bq      (scale folded into weights)
  KT[e,t] = Wk^T-matmul
  V [t,e] = Wv^T-matmul, stored head-major with a ones column per head
  per head h, per q-block (512 wide):
    for k-tile (128 rows, causally live only):
      scoresT[k,q] = KT_h tile^T-matmul QT_h      (PSUM, fp32)
      p = exp(scoresT)                            (ScalarE, -> bf16 SBUF)
      causal zero-fill on the diagonal tile       (GpSimd affine_select)
      acc[d+1, q] += [V_h | 1]^T-matmul p         (PSUM accumulate)
    row d of acc = softmax denominator; rows 0..63 = unnormalized (PV)^T
  normalize with one batched reciprocal + broadcast multiply
  outT[e,t] = Wo^T-matmul A^T + (bo + Wo@bv)     (bv folded: softmax rows sum to 1)

Host glue transposes X/W (bf16) on the way in and out^T back on the way out.
"""

import numpy as np
import ml_dtypes

# Problem constants (hardcoded per the harness contract).
NUM_SEQS = 8
SEQ = 1024
EMBED = 1024
HEADS = 16
HEAD_DIM = 64
P = 128
NK = EMBED // P          # 8 contraction tiles
QB = 512                 # q-block width
NQB = SEQ // QB          # 2 q-blocks
HV = HEAD_DIM + 1        # V columns per head incl. ones column

_CACHE = {}


def build_module(reps=1):
    """Build and compile the SPMD Bass module. reps>1 wraps the body in a
    hardware loop (used only for wall-clock timing in test harnesses)."""
    import concourse.mybir as mybir
    import concourse.tile as tile
    from concourse import bacc
    from contextlib import ExitStack

    bf16 = mybir.dt.bfloat16
    f32 = mybir.dt.float32
    EXP = mybir.ActivationFunctionType.Exp

    nc = bacc.Bacc("TRN2", target_bir_lowering=False, debug=False,
                   num_devices=NUM_SEQS)

    xt_d = nc.dram_tensor("xt", [EMBED, SEQ], bf16, kind="ExternalInput").ap()
    wq_d = nc.dram_tensor("wqt", [EMBED, EMBED], bf16, kind="ExternalInput").ap()
    wk_d = nc.dram_tensor("wkt", [EMBED, EMBED], bf16, kind="ExternalInput").ap()
    wv_d = nc.dram_tensor("wvt", [EMBED, EMBED], bf16, kind="ExternalInput").ap()
    wo_d = nc.dram_tensor("wot", [EMBED, EMBED], bf16, kind="ExternalInput").ap()
    bq_d = nc.dram_tensor("bqs", [EMBED], f32, kind="ExternalInput").ap()
    bo_d = nc.dram_tensor("boe", [EMBED], f32, kind="ExternalInput").ap()
    ot_d = nc.dram_tensor("ot", [EMBED, SEQ], f32, kind="ExternalOutput").ap()

    xt_v = xt_d.rearrange("(a p) t -> a p t", p=P)
    wq_v = wq_d.rearrange("(a p) e -> a p e", p=P)
    wk_v = wk_d.rearrange("(a p) e -> a p e", p=P)
    wv_v = wv_d.rearrange("(a p) e -> a p e", p=P)
    wo_v = wo_d.rearrange("(a p) e -> a p e", p=P)

    with tile.TileContext(nc) as tc:
        with ExitStack() as ctx:
            const = ctx.enter_context(tc.tile_pool(name="const", bufs=1))
            pp_mm = ctx.enter_context(
                tc.tile_pool(name="pp_mm", bufs=2, space="PSUM"))
            pp_sc = ctx.enter_context(
                tc.tile_pool(name="pp_sc", bufs=2, space="PSUM"))
            pp_av = ctx.enter_context(
                tc.tile_pool(name="pp_av", bufs=2, space="PSUM"))
            pexp = ctx.enter_context(tc.tile_pool(name="pexp", bufs=4))
            postg = ctx.enter_context(tc.tile_pool(name="postg", bufs=3))

            def body(_it=None):
                # --- persistent SBUF tensors ------------------------------
                wq = [const.tile([P, EMBED], bf16, tag=f"wq{k}", name=f"wq{k}") for k in range(NK)]
                wk = [const.tile([P, EMBED], bf16, tag=f"wk{k}", name=f"wk{k}") for k in range(NK)]
                wv = [const.tile([P, EMBED], bf16, tag=f"wv{k}", name=f"wv{k}") for k in range(NK)]
                wo = [const.tile([P, EMBED], bf16, tag=f"wo{k}", name=f"wo{k}") for k in range(NK)]
                xt = [const.tile([P, SEQ], bf16, tag=f"xt{k}", name=f"xt{k}") for k in range(NK)]
                qt = [const.tile([P, SEQ], bf16, tag=f"qt{a}", name=f"qt{a}") for a in range(NK)]
                kt = [const.tile([P, SEQ], bf16, tag=f"kt{a}", name=f"kt{a}") for a in range(NK)]
                vv = [const.tile([P, HEADS * HV], bf16, tag=f"vv{m}", name=f"vv{m}")
                      for m in range(NK)]
                at = [const.tile([P, SEQ], bf16, tag=f"at{a}", name=f"at{a}") for a in range(NK)]
                bqs = const.tile([P, NK], f32, tag="bqs")
                boe = const.tile([P, NK], f32, tag="boe")
                den = const.tile([HEADS, SEQ], f32, tag="den")
                rec = const.tile([HEADS, SEQ], f32, tag="rec")
                # sel2[i, c] = 1.0 iff c // 64 == i  (i in {0,1}) -- used to
                # broadcast per-head reciprocal denominators across the 64
                # partitions of each head's rows via a tiny PE matmul.
                sel2 = const.tile([2, P], f32, tag="sel2")
                nc.vector.memset(sel2, 1.0)
                nc.gpsimd.affine_select(
                    out=sel2, in_=sel2, compare_op=mybir.AluOpType.is_ge,
                    fill=0.0, base=0, pattern=[[1, P]],
                    channel_multiplier=-HEAD_DIM)
                nc.gpsimd.affine_select(
                    out=sel2, in_=sel2, compare_op=mybir.AluOpType.is_ge,
                    fill=0.0, base=HEAD_DIM - 1, pattern=[[-1, P]],
                    channel_multiplier=HEAD_DIM)

                # --- loads ------------------------------------------------
                nc.sync.dma_start(out=bqs, in_=bq_d.rearrange("(a p) -> p a", p=P))
                nc.sync.dma_start(out=boe, in_=bo_d.rearrange("(a p) -> p a", p=P))
                for k in range(NK):
                    nc.sync.dma_start(out=xt[k], in_=xt_v[k])
                    nc.sync.dma_start(out=wq[k], in_=wq_v[k])
                    nc.sync.dma_start(out=wk[k], in_=wk_v[k])
                    nc.sync.dma_start(out=wv[k], in_=wv_v[k])
                    nc.sync.dma_start(out=wo[k], in_=wo_v[k])
                for m in range(NK):
                    # ones column per head for the fused denominator
                    nc.vector.memset(
                        vv[m].rearrange("p (h c) -> p h c", c=HV)[:, :, HEAD_DIM:HV],
                        1.0)

                # --- projections ------------------------------------------
                # QT[e,t], KT[e,t]: lhsT = W^T tile [c,e], rhs = X^T [c,t]
                for a in range(NK):
                    for n in range(NQB):
                        ts = slice(n * QB, (n + 1) * QB)
                        es = slice(a * P, (a + 1) * P)
                        ps = pp_mm.tile([P, QB], f32)
                        for k in range(NK):
                            nc.tensor.matmul(ps, lhsT=wq[k][:, es],
                                             rhs=xt[k][:, ts],
                                             start=(k == 0), stop=(k == NK - 1))
                        nc.vector.tensor_scalar(
                            out=qt[a][:, ts], in0=ps,
                            scalar1=bqs[:, a:a + 1], scalar2=None,
                            op0=mybir.AluOpType.add)
                        ps = pp_mm.tile([P, QB], f32)
                        for k in range(NK):
                            nc.tensor.matmul(ps, lhsT=wk[k][:, es],
                                             rhs=xt[k][:, ts],
                                             start=(k == 0), stop=(k == NK - 1))
                        nc.vector.tensor_copy(out=kt[a][:, ts], in_=ps)
                # V[t,e]: lhsT = X^T tile [c,t], rhs = Wv^T [c,e]
                for m in range(NK):
                    ms = slice(m * P, (m + 1) * P)
                    for n in range(NQB):
                        es = slice(n * QB, (n + 1) * QB)
                        ps = pp_mm.tile([P, QB], f32)
                        for k in range(NK):
                            nc.tensor.matmul(ps, lhsT=xt[k][:, ms],
                                             rhs=wv[k][:, es],
                                             start=(k == 0), stop=(k == NK - 1))
                        # scatter heads into the HV-strided layout
                        nc.vector.tensor_copy(
                            out=vv[m][:, n * 8 * HV:(n + 1) * 8 * HV].rearrange(
                                "p (h c) -> p h c", c=HV)[:, :, 0:HEAD_DIM],
                            in_=ps.rearrange("p (h c) -> p h c", c=HEAD_DIM))

                # --- attention --------------------------------------------
                for h in range(HEADS):
                    a_h = h // 2
                    po = (h % 2) * HEAD_DIM
                    hvs = slice(h * HV, h * HV + HV)
                    for qb in range(NQB):
                        acc = pp_av.tile([P, QB], f32)
                        nkb = (qb + 1) * (QB // P)
                        for kb in range(nkb):
                            c0 = max(0, kb * P - qb * QB)
                            sc = pp_sc.tile([P, QB], f32)
                            nc.tensor.matmul(
                                sc[:, c0:QB],
                                lhsT=kt[a_h][po:po + HEAD_DIM,
                                             kb * P:(kb + 1) * P],
                                rhs=qt[a_h][po:po + HEAD_DIM,
                                            qb * QB + c0:(qb + 1) * QB],
                                start=True, stop=True)
                            pt = pexp.tile([P, QB], bf16)
                            nc.scalar.activation(out=pt[:, c0:QB],
                                                 in_=sc[:, c0:QB], func=EXP)
                            if kb >= qb * (QB // P):
                                # diagonal tile: zero strictly-upper triangle
                                nc.gpsimd.affine_select(
                                    out=pt[:, c0:c0 + P], in_=pt[:, c0:c0 + P],
                                    compare_op=mybir.AluOpType.is_ge,
                                    fill=0.0, base=0,
                                    pattern=[[1, P]], channel_multiplier=-1)
                            nc.tensor.matmul(
                                acc[:HV, c0:QB], lhsT=vv[kb][:, hvs],
                                rhs=pt[:, c0:QB],
                                start=(kb == 0), stop=(kb == nkb - 1))
                        qs = slice(qb * QB, (qb + 1) * QB)
                        # engines can only address partitions {0,32,64,96};
                        # bounce the denominator row via SBUF->SBUF DMA.
                        ds = postg.tile([1, QB], f32, name="ds", tag="ds")
                        nc.vector.tensor_copy(out=ds, in_=acc[HEAD_DIM:HV, :])
                        nc.sync.dma_start(out=den[h:h + 1, qs], in_=ds)
                        nc.vector.tensor_copy(out=at[a_h][po:po + HEAD_DIM, qs],
                                              in_=acc[0:HEAD_DIM, :])

                # --- normalize --------------------------------------------
                nc.vector.reciprocal(out=rec, in_=den)
                for a in range(NK):
                    pr = pexp.tile([2, SEQ], f32, name="pr", tag="pr")
                    nc.sync.dma_start(out=pr, in_=rec[2 * a:2 * a + 2, :])
                    for qb in range(NQB):
                        qs = slice(qb * QB, (qb + 1) * QB)
                        bc = pp_sc.tile([P, QB], f32, name="bc")
                        nc.tensor.matmul(bc, lhsT=sel2, rhs=pr[:, qs],
                                         start=True, stop=True)
                        nc.vector.tensor_mul(at[a][:, qs], at[a][:, qs], bc)

                # --- output projection ------------------------------------
                for m in range(NK):
                    for n in range(NQB):
                        ts = slice(n * QB, (n + 1) * QB)
                        ps = pp_mm.tile([P, QB], f32)
                        for k in range(NK):
                            nc.tensor.matmul(ps,
                                             lhsT=wo[k][:, m * P:(m + 1) * P],
                                             rhs=at[k][:, ts],
                                             start=(k == 0), stop=(k == NK - 1))
                        ob = postg.tile([P, QB], f32)
                        nc.vector.tensor_scalar(
                            out=ob, in0=ps, scalar1=boe[:, m:m + 1],
                            scalar2=None, op0=mybir.AluOpType.add)
                        nc.sync.dma_start(
                            out=ot_d[m * P:(m + 1) * P, ts], in_=ob)

            if reps == 1:
                body()
            else:
                with tc.For_i(0, reps, 1) as it:
                    body(it)

    nc.compile()
    return nc


def _get_module(reps=1):
    key = ("nc", reps)
    if key not in _CACHE:
        _CACHE[key] = build_module(reps)
    return _CACHE[key]


def _prep_inputs(hidden_states, Wq, bq, Wk, Wv, bv, Wo, bo):
    bf16 = ml_dtypes.bfloat16
    f32 = np.float32
    scale = f32(1.0) / f32(np.sqrt(HEAD_DIM))
    wqt = np.ascontiguousarray((Wq * scale).T).astype(bf16)
    wkt = np.ascontiguousarray(Wk.T).astype(bf16)
    wvt = np.ascontiguousarray(Wv.T).astype(bf16)
    wot = np.ascontiguousarray(Wo.T).astype(bf16)
    bqs = (bq * scale).astype(f32)
    boe = (bo + Wo.astype(f32) @ bv.astype(f32)).astype(f32)
    shared = dict(wqt=wqt, wkt=wkt, wvt=wvt, wot=wot, bqs=bqs, boe=boe)
    in_maps = []
    for i in range(NUM_SEQS):
        xs = hidden_states[i * SEQ:(i + 1) * SEQ, :]
        xt = np.ascontiguousarray(xs.T).astype(bf16)
        in_maps.append(dict(shared, xt=xt))
    return in_maps


def _numpy_fallback(hidden_states, seq_len, Wq, bq, Wk, Wv, bv, Wo, bo):
    # Generic ragged reference (only used if seq_len deviates from 8x1024).
    T = hidden_states.shape[0]
    q = (hidden_states @ Wq.T + bq).reshape(T, HEADS, HEAD_DIM)
    k = (hidden_states @ Wk.T).reshape(T, HEADS, HEAD_DIM)
    v = (hidden_states @ Wv.T + bv).reshape(T, HEADS, HEAD_DIM)
    sl = np.asarray(seq_len).astype(np.int64)
    cu = np.concatenate([[0], np.cumsum(sl)])
    out = np.empty((T, HEADS * HEAD_DIM), np.float32)
    scale = 1.0 / np.float32(np.sqrt(HEAD_DIM))
    for b in range(len(sl)):
        s, e = int(cu[b]), int(cu[b + 1])
        qb, kb, vb = q[s:e], k[s:e], v[s:e]
        sc = np.einsum("qhd,khd->hqk", qb, kb) * scale
        L = e - s
        mask = np.tril(np.ones((L, L), bool))
        sc = np.where(mask[None], sc, -np.inf)
        sc = sc - sc.max(-1, keepdims=True)
        p = np.exp(sc)
        p /= p.sum(-1, keepdims=True)
        ob = np.einsum("hqk,khd->qhd", p, vb)
        out[s:e] = ob.reshape(L, -1)
    return (out @ Wo.T + bo).astype(np.float32)


def kernel(hidden_states, seq_len, Wq, bq, Wk, Wv, bv, Wo, bo):
    hidden_states = np.asarray(hidden_states, dtype=np.float32)
    seq_len = np.asarray(seq_len)
    Wq, bq = np.asarray(Wq, np.float32), np.asarray(bq, np.float32)
    Wk = np.asarray(Wk, np.float32)
    Wv, bv = np.asarray(Wv, np.float32), np.asarray(bv, np.float32)
    Wo, bo = np.asarray(Wo, np.float32), np.asarray(bo, np.float32)

    if (seq_len.shape != (NUM_SEQS,) or not np.all(seq_len == SEQ)
            or hidden_states.shape != (NUM_SEQS * SEQ, EMBED)):
        return _numpy_fallback(hidden_states, seq_len, Wq, bq, Wk, Wv, bv,
                               Wo, bo)

    from concourse.bass_utils import run_bass_kernel_spmd

    nc = _get_module(reps=1)
    in_maps = _prep_inputs(hidden_states, Wq, bq, Wk, Wv, bv, Wo, bo)
    res = run_bass_kernel_spmd(nc, in_maps, list(range(NUM_SEQS)))
    out = np.empty((NUM_SEQS * SEQ, EMBED), np.float32)
    for i in range(NUM_SEQS):
        out[i * SEQ:(i + 1) * SEQ, :] = res.results[i]["ot"].T
    return out
